# revision 70
# baseline (speedup 1.0000x reference)
"""Causal self-attention (B=2, T=2048, C=1024, H=16, rope) on 8 trn2 cores.

Sharding: core i = (batch b = i // 4, head-group g = i % 4 owning heads 4g..4g+3).
Each core computes its 4 heads' attention and a partial projection (transposed);
the host sums the 4 head-group partials per batch and adds b_proj.

Dataflow (all matmuls bf16; x is uploaded pre-transposed in bf16 so no
on-chip transposes are needed):
  xT <- DMA of host-side x.T                                        [C, Tch]
  qk^T = Wqk.T @ xT; eviction fuses the per-partition bias on DVE
    (scalar_tensor_tensor)                                           [128, Tch]
  rope: 1 Pool mul + 4 DVE muls + 2 DVE adds per head-tile
    -> qT[64,h,T], kT[64,h,T] (layout per head: [q_e|q_o] / [k_e|k_o])
  v = xT.T @ Wv (+bias via K=1 matmul), natural [Tch, 4*64+ones] bf16
  S^T[k,q] per (head-pair, k-slab) in [128,2,512] PSUM; causal diagonal
    handled by accumulating a -1e9 triangle via an identity x tri matmul
  expS = exp(S^T/8) -> bf16 SBUF (one Act op per head-pair-slab);
    p=0 tiles are parity-double-buffered so the next chunk's S/exp can
    start while the current chunk's PV still reads them
  [O^T; l] = sum_j [v|1].T-slab @ expS   (PV + denominators together)
  yT = O^T * (1/l)  (DVE recip from PSUM, gpsimd partition_broadcast)
  out^T = Wp-slab.T @ yT -> bf16 -> DMA out (SP queue)

Software-pipelined emission: chunk tcH's attention overlaps chunk tcH+1's
projection phase; proj(tcH) is deferred into iteration tcH+1; PE p-state
is warmed with dummy matmuls so real work runs at full clock.
"""

import numpy as np

B, T, C, H = 2, 2048, 1024, 16
HS = C // H            # 64
HPC = H // 4           # 4 heads per core
NCORES = 8
TCH = 512              # t/q chunk size
NCH = T // TCH         # 4 chunks
NSLAB = T // 128       # 16 t-slabs

_cache = {}
last_results = None    # BassKernelResults of the most recent run (for test.py)


def _build():
    import concourse.bacc as bacc
    import concourse.mybir as mybir
    import concourse.tile as tile
    from concourse.masks import make_identity

    F32 = mybir.dt.float32
    F32R = mybir.dt.float32r
    BF16 = mybir.dt.bfloat16
    FP8 = mybir.dt.float8e4
    DR = mybir.MatmulPerfMode.DoubleRow
    AF = mybir.ActivationFunctionType

    nc = bacc.Bacc("TRN2", target_bir_lowering=False, debug=False,
                   num_devices=NCORES)

    x_t = nc.dram_tensor("x_t", (C, T), BF16, kind="ExternalInput")
    wqk = nc.dram_tensor("wqk", (C, 512), BF16, kind="ExternalInput")
    bqk = nc.dram_tensor("bqk", (128, 4), F32, kind="ExternalInput")
    wv = nc.dram_tensor("wv", (C, 256), BF16, kind="ExternalInput")
    bvr = nc.dram_tensor("bvr", (1, 256), BF16, kind="ExternalInput")
    wp = nc.dram_tensor("wp", (256, C), BF16, kind="ExternalInput")
    cos_in = nc.dram_tensor("cos_in", (128, T), BF16, kind="ExternalInput")
    sin_in = nc.dram_tensor("sin_in", (128, T), BF16, kind="ExternalInput")
    tri_in = nc.dram_tensor("tri_in", (128, 128), BF16, kind="ExternalInput")
    out_t = nc.dram_tensor("out_t", (C, T), BF16, kind="ExternalOutput")

    with tile.TileContext(nc) as tc:
        with (
            tc.tile_pool(name="const", bufs=1) as const,
            tc.tile_pool(name="work", bufs=3) as work,
            tc.tile_pool(name="ytp", bufs=2) as ytp,
            tc.tile_pool(name="ost", bufs=6) as ost,
            tc.tile_pool(name="ps_a", bufs=2, space="PSUM") as ps_a,
            tc.tile_pool(name="ps_s", bufs=2, space="PSUM") as ps_s,
            tc.tile_pool(name="ps_o", bufs=2, space="PSUM") as ps_o,
        ):
            # ---- constants / weights (x chunk 0 first so compute starts early) ----
            ident = const.tile([128, 128], BF16)
            make_identity(nc, ident)
            ones_h = const.tile([1, 128], BF16)
            nc.gpsimd.memset(ones_h[:], 1.0)
            ones5 = const.tile([128, TCH], BF16)
            nc.gpsimd.memset(ones5[:], 1.0)

            # PE p-state warmup: dependency-free dummy matmuls from t~0 so the
            # 3us ramp to full clock completes before real work arrives
            for w in range(40):
                pw = ps_o.tile([1, TCH], F32, tag="o", name=f"warm{w}")
                nc.tensor.matmul(pw[:], ones5[0:128, 0:1], ones5[:],
                                 start=True, stop=True)

            xT_db = [[const.tile([128, 4, TCH], BF16, name=f"xT{i}_{hf}")
                      for hf in range(2)] for i in range(2)]

            def emit_dma_xt(tcH):
                t0 = tcH * TCH
                for hf in range(2):
                    nc.sync.dma_start(
                        xT_db[tcH % 2][hf][:],
                        x_t[hf * 512:(hf + 1) * 512, t0:t0 + TCH]
                        .rearrange("(s p) t -> p s t", p=128))

            wqk_sb = const.tile([128, 8, 512], BF16)
            nc.sync.dma_start(
                wqk_sb[:, 0:4, :],
                wqk.ap()[0:512, :].rearrange("(s p) m -> p s m", p=128))
            emit_dma_xt(0)
            nc.sync.dma_start(
                wqk_sb[:, 4:8, :],
                wqk.ap()[512:1024, :].rearrange("(s p) m -> p s m", p=128))
            # spread remaining weight loads across idle DGE queues
            cos_sb = const.tile([128, T], BF16)
            nc.scalar.dma_start(cos_sb[:], cos_in[:, :])
            sin_sb = const.tile([128, T], BF16)
            nc.scalar.dma_start(sin_sb[:], sin_in[:, :])
            bqk_sb = const.tile([128, 4], F32)
            nc.scalar.dma_start(bqk_sb[:], bqk[:, :])
            wv_sb = const.tile([128, 8, 256], BF16)
            nc.gpsimd.dma_start(wv_sb[:],
                                wv.ap().rearrange("(s p) m -> p s m", p=128))
            bvr_sb = const.tile([1, 256], BF16)
            nc.gpsimd.dma_start(bvr_sb[:], bvr[:, :])
            tri_sb = const.tile([128, 128], BF16)
            nc.gpsimd.dma_start(tri_sb[:], tri_in[:, :])
            wp_sb = const.tile([128, 2, 1024], BF16)
            nc.gpsimd.dma_start(wp_sb[:],
                                wp.ap().rearrange("(s p) m -> p s m", p=128))

            # ---- persistent activations ----
            qT_sb = const.tile([64, HPC, T], BF16, name="qT")
            kT_sb = const.tile([64, HPC, T], BF16, name="kT")
            v_sb = const.tile([128, NSLAB, HPC, 65], BF16, name="v")
            nc.vector.memset(v_sb[:, :, :, 64], 1.0)  # ones col, stays forever
            # p=0 tiles are parity-double-buffered so chunk tcH+1's S/exp
            # (p=0) can run while chunk tcH's PV still reads its p=0 tiles
            expS0 = [[const.tile([128, 2, TCH], BF16, name=f"eS0{a}_{j}")
                      for j in range(NSLAB)] for a in range(2)]
            expS1 = [const.tile([128, 2, TCH], BF16, name=f"eS1_{j}")
                     for j in range(NSLAB)]

            def expS(tcH, p):
                return expS0[tcH % 2] if p == 0 else expS1

            def emit_qkv(tcH):
                xTh = xT_db[tcH % 2]
                tcols = slice(tcH * TCH, (tcH + 1) * TCH)
                for m in range(HPC):
                    pqk = ps_a.tile([128, TCH], F32, tag="a")
                    for s in range(8):
                        nc.tensor.matmul(pqk[:],
                                         wqk_sb[:, s, m * 128:(m + 1) * 128],
                                         xTh[s // 4][:, s % 4, :],
                                         start=(s == 0), stop=(s == 7))
                    tQ = work.tile([128, TCH], BF16, tag="tQ")
                    nc.vector.scalar_tensor_tensor(
                        tQ[:], pqk[:], bqk_sb[:, m:m + 1], ones5[:],
                        op0=mybir.AluOpType.add, op1=mybir.AluOpType.mult)
                    tA = work.tile([128, TCH], BF16, tag="tA")
                    tB = work.tile([128, TCH], BF16, tag="tB")
                    nc.gpsimd.tensor_mul(tA[:], tQ[:], cos_sb[:, tcols])
                    for (a0, b0) in ((0, 32), (32, 0), (64, 96), (96, 64)):
                        nc.vector.tensor_mul(tB[a0:a0 + 32, :], tQ[b0:b0 + 32, :],
                                             sin_sb[b0:b0 + 32, tcols])
                    nc.vector.tensor_add(qT_sb[:, m, tcols],
                                         tA[0:64, :], tB[0:64, :])
                    nc.vector.tensor_add(kT_sb[:, m, tcols],
                                         tA[64:128, :], tB[64:128, :])
                for ts in range(4):
                    pv = ps_a.tile([128, 256], F32, tag="a")
                    for s in range(8):
                        nc.tensor.matmul(pv[:],
                                         xTh[s // 4][:, s % 4,
                                                     ts * 128:(ts + 1) * 128],
                                         wv_sb[:, s, :], start=(s == 0),
                                         stop=False)
                    nc.tensor.matmul(pv[:], ones_h[0:1, :], bvr_sb[:],
                                     start=False, stop=True,
                                     skip_group_check=True)
                    sl = tcH * 4 + ts
                    nc.scalar.activation(
                        v_sb[:, sl, :, 0:64],
                        pv[:].rearrange("p (h e) -> p h e", e=64), AF.Copy)

            def emit_s_exp(tcH, p):
                nslabs = 4 * tcH + 4
                for j in range(nslabs):
                    rr = j - 4 * tcH
                    r = max(rr, 0) * 128
                    qs = slice(tcH * TCH + r, (tcH + 1) * TCH)
                    psS = ps_s.tile([128, 2, TCH], F32, tag="S")
                    for hh in range(2):
                        h = 2 * p + hh
                        nc.tensor.matmul(psS[:, hh, r:TCH],
                                         kT_sb[:, h, j * 128:(j + 1) * 128],
                                         qT_sb[:, h, qs],
                                         start=True, stop=(rr < 0))
                        if rr >= 0:
                            nc.tensor.matmul(psS[:, hh, r:r + 128], ident[:],
                                             tri_sb[:], start=False, stop=True,
                                             skip_group_check=True)
                    eS = expS(tcH, p)[j]
                    nc.scalar.activation(eS[:, :, r:TCH], psS[:, :, r:TCH],
                                         AF.Exp, scale=0.125)

            def emit_pv(tcH, yT_ch, p):
                yT_p = yT_ch[p]
                nslabs = 4 * tcH + 4
                for hh in range(2):
                    h = 2 * p + hh
                    po = ps_o.tile([65, TCH], F32, tag="o")
                    for j in range(nslabs):
                        rr = j - 4 * tcH
                        r = max(rr, 0) * 128
                        nc.tensor.matmul(po[:, r:TCH],
                                         v_sb[:, j, h, :],
                                         expS(tcH, p)[j][:, hh, r:TCH],
                                         start=(j == 0),
                                         stop=(j == nslabs - 1))
                    l_r = work.tile([1, TCH], BF16, tag="lr")
                    with nc.allow_low_precision(reason="bf16 1/l"):
                        nc.vector.reciprocal(l_r[:], po[64:65, :])
                    l_b = work.tile([64, TCH], BF16, tag="lb")
                    nc.gpsimd.partition_broadcast(l_b[:], l_r[0:1, :])
                    nc.vector.tensor_mul(yT_p[64 * hh:64 * hh + 64, :],
                                         po[0:64, :], l_b[:])

            def emit_proj(tcH, yT_ch, order=(0, 1), pool=None):
                tcols = slice(tcH * TCH, (tcH + 1) * TCH)
                for ct in range(8):
                    pp = (pool or ps_o).tile([128, TCH], F32,
                                             tag="S" if pool else "o")
                    for i, s in enumerate(order):
                        nc.tensor.matmul(pp[:],
                                         wp_sb[:, s, ct * 128:(ct + 1) * 128],
                                         yT_ch[s][:, :], start=(i == 0),
                                         stop=(i == 1))
                    o_st = ost.tile([128, TCH], BF16, tag="ost")
                    nc.vector.tensor_copy(o_st[:], pp[:])
                    nc.sync.dma_start(out_t[ct * 128:(ct + 1) * 128, tcols],
                                      o_st[:])

            # ---- software-pipelined emission ----
            emit_qkv(0)

            pending_proj = None   # (tcH, yT_ch) deferred into next iteration
            for tcH in range(NCH):
                if tcH + 1 < NCH:
                    emit_dma_xt(tcH + 1)
                if tcH == 0:
                    emit_s_exp(0, 0)
                if pending_proj is not None:
                    emit_proj(*pending_proj)
                emit_s_exp(tcH, 1)
                if tcH + 1 < NCH:
                    emit_qkv(tcH + 1)
                    emit_s_exp(tcH + 1, 0)
                yT_ch = [ytp.tile([128, TCH], BF16, tag=f"yT{s}",
                                  name=f"yT{s}") for s in range(2)]
                emit_pv(tcH, yT_ch, 0)
                emit_pv(tcH, yT_ch, 1)
                pending_proj = (tcH, yT_ch)
            emit_proj(*pending_proj, pool=ps_s)

    nc.compile()
    return nc


def _rope_tables():
    pos = np.arange(T, dtype=np.float32)[:, None]                  # [T, 1]
    i = np.arange(1, HS // 2 + 1, dtype=np.float32)[None]          # [1, 32]
    theta = 1.0 / 10000.0 ** (2.0 * (i - 1.0) / HS)
    ang = pos * theta                                              # [T, 32]
    cos, sin = np.cos(ang).T, np.sin(ang).T                        # [32, T]
    cos_rep = np.tile(cos, (4, 1)).astype(np.float32)              # [128, T]
    sin_sgn = np.concatenate([sin, -sin, sin, -sin], 0).astype(np.float32)
    return cos_rep, sin_sgn


def kernel(x, W_qkv, b_qkv, W_proj, b_proj):
    global last_results
    import ml_dtypes
    from concourse.bass_utils import run_bass_kernel_spmd

    BF = ml_dtypes.bfloat16
    F8 = ml_dtypes.float8_e4m3

    def fold(a):
        # [C, N] -> [128, 4, 2, N]: dim (s, j, k) covers c = 256 s + 128 j + k
        n = a.shape[1]
        return np.ascontiguousarray(
            a.reshape(4, 2, 128, n).transpose(2, 0, 1, 3))

    if "nc" not in _cache:
        _cache["nc"] = _build()
    nc = _cache["nc"]

    x = np.asarray(x, np.float32)
    W_qkv = np.asarray(W_qkv, np.float32)
    b_qkv = np.asarray(b_qkv, np.float32)
    W_proj = np.asarray(W_proj, np.float32)
    b_proj = np.asarray(b_proj, np.float32)

    cos_rep, sin_sgn = _rope_tables()
    tri = np.tril(np.full((128, 128), -1e9, dtype=np.float32), -1)

    in_maps = []
    for core in range(NCORES):
        b, g = core // 4, core % 4
        heads = [4 * g + j for j in range(HPC)]
        qk_cols, bqk_cols = [], []
        for h in heads:
            Wq = W_qkv[:, h * 3 * HS:h * 3 * HS + HS]
            Wk = W_qkv[:, h * 3 * HS + HS:h * 3 * HS + 2 * HS]
            bq = b_qkv[h * 3 * HS:h * 3 * HS + HS]
            bk = b_qkv[h * 3 * HS + HS:h * 3 * HS + 2 * HS]
            qk_cols.append(np.concatenate(
                [Wq[:, 0::2], Wq[:, 1::2], Wk[:, 0::2], Wk[:, 1::2]], axis=1))
            bqk_cols.append(np.concatenate(
                [bq[0::2], bq[1::2], bk[0::2], bk[1::2]]))
        wqk_core = np.concatenate(qk_cols, axis=1)                 # [C, 512]
        bqk_core = np.stack(bqk_cols, axis=1)                      # [128, 4]
        wv_core = np.concatenate(
            [W_qkv[:, h * 3 * HS + 2 * HS:h * 3 * HS + 3 * HS] for h in heads],
            axis=1)                                                # [C, 256]
        bv_core = np.concatenate(
            [b_qkv[h * 3 * HS + 2 * HS:h * 3 * HS + 3 * HS] for h in heads])

        in_maps.append({
            "x_t": np.ascontiguousarray(x[b].T).astype(BF),
            "wqk": np.ascontiguousarray(wqk_core).astype(BF),
            "bqk": np.ascontiguousarray(bqk_core),
            "wv": np.ascontiguousarray(wv_core).astype(BF),
            "bvr": np.ascontiguousarray(bv_core[None, :]).astype(BF),
            "wp": np.ascontiguousarray(W_proj[g * 256:(g + 1) * 256, :]).astype(BF),
            "cos_in": cos_rep.astype(BF),
            "sin_in": sin_sgn.astype(BF),
            "tri_in": tri.astype(BF),
        })

    res = run_bass_kernel_spmd(nc, in_maps, core_ids=list(range(NCORES)))
    last_results = res

    out = np.zeros((B, T, C), dtype=np.float32)
    for core in range(NCORES):
        b = core // 4
        out[b] += res.results[core]["out_t"].astype(np.float32).T
    out += b_proj[None, None, :]
    return out


# revision 76
# speedup vs baseline: 1.0032x; 1.0032x over previous
"""Causal self-attention (B=2, T=2048, C=1024, H=16, rope) on 8 trn2 cores.

Sharding: core i = (batch b = i // 4, head-group g = i % 4 owning heads 4g..4g+3).
Each core computes its 4 heads' attention and a partial projection (transposed);
the host sums the 4 head-group partials per batch and adds b_proj.

Dataflow (all matmuls bf16; x is uploaded pre-transposed in bf16 so no
on-chip transposes are needed):
  xT <- DMA of host-side x.T                                        [C, Tch]
  qk^T = Wqk.T @ xT; eviction fuses the per-partition bias on DVE
    (scalar_tensor_tensor)                                           [128, Tch]
  rope: 1 Pool mul + 4 DVE muls + 2 DVE adds per head-tile
    -> qT[64,h,T], kT[64,h,T] (layout per head: [q_e|q_o] / [k_e|k_o])
  v = xT.T @ Wv (+bias via K=1 matmul), natural [Tch, 4*64+ones] bf16
  S^T[k,q] per (head-pair, k-slab) in [128,2,512] PSUM; causal diagonal
    handled by accumulating a -1e9 triangle via an identity x tri matmul
  expS = exp(S^T/8) -> bf16 SBUF (one Act op per head-pair-slab);
    p=0 tiles are parity-double-buffered so the next chunk's S/exp can
    start while the current chunk's PV still reads them
  [O^T; l] = sum_j [v|1].T-slab @ expS   (PV + denominators together)
  yT = O^T * (1/l)  (DVE recip from PSUM, gpsimd partition_broadcast)
  out^T = Wp-slab.T @ yT -> bf16 -> DMA out (SP queue)

Software-pipelined emission: chunk tcH's attention overlaps chunk tcH+1's
projection phase; proj(tcH) is deferred into iteration tcH+1; PE p-state
is warmed with dummy matmuls so real work runs at full clock.
"""

import numpy as np

B, T, C, H = 2, 2048, 1024, 16
HS = C // H            # 64
HPC = H // 4           # 4 heads per core
NCORES = 8
TCH = 512              # t/q chunk size
NCH = T // TCH         # 4 chunks
NSLAB = T // 128       # 16 t-slabs

_cache = {}
last_results = None    # BassKernelResults of the most recent run (for test.py)


def _build():
    import concourse.bacc as bacc
    import concourse.mybir as mybir
    import concourse.tile as tile
    from concourse.masks import make_identity

    F32 = mybir.dt.float32
    F32R = mybir.dt.float32r
    BF16 = mybir.dt.bfloat16
    FP8 = mybir.dt.float8e4
    DR = mybir.MatmulPerfMode.DoubleRow
    AF = mybir.ActivationFunctionType

    nc = bacc.Bacc("TRN2", target_bir_lowering=False, debug=False,
                   num_devices=NCORES)

    x_t = nc.dram_tensor("x_t", (C, T), BF16, kind="ExternalInput")
    wqk = nc.dram_tensor("wqk", (C, 512), BF16, kind="ExternalInput")
    bqk = nc.dram_tensor("bqk", (128, 4), F32, kind="ExternalInput")
    wv = nc.dram_tensor("wv", (C, 256), BF16, kind="ExternalInput")
    bvr = nc.dram_tensor("bvr", (1, 256), BF16, kind="ExternalInput")
    wp = nc.dram_tensor("wp", (256, C), BF16, kind="ExternalInput")
    cos_in = nc.dram_tensor("cos_in", (128, T), BF16, kind="ExternalInput")
    sin_in = nc.dram_tensor("sin_in", (128, T), BF16, kind="ExternalInput")
    tri_in = nc.dram_tensor("tri_in", (128, 128), BF16, kind="ExternalInput")
    out_t = nc.dram_tensor("out_t", (C, T), BF16, kind="ExternalOutput")

    with tile.TileContext(nc) as tc:
        with (
            tc.tile_pool(name="const", bufs=1) as const,
            tc.tile_pool(name="work", bufs=3) as work,
            tc.tile_pool(name="ytp", bufs=2) as ytp,
            tc.tile_pool(name="ost", bufs=6) as ost,
            tc.tile_pool(name="ps_a", bufs=2, space="PSUM") as ps_a,
            tc.tile_pool(name="ps_s", bufs=2, space="PSUM") as ps_s,
            tc.tile_pool(name="ps_o", bufs=2, space="PSUM") as ps_o,
        ):
            # ---- constants / weights (x chunk 0 first so compute starts early) ----
            ident = const.tile([128, 128], BF16)
            make_identity(nc, ident)
            ones_h = const.tile([1, 128], BF16)
            nc.gpsimd.memset(ones_h[:], 1.0)
            ones5 = const.tile([128, TCH], BF16)
            nc.gpsimd.memset(ones5[:], 1.0)

            # PE p-state warmup: dependency-free dummy matmuls from t~0 so the
            # 3us ramp to full clock completes before real work arrives
            for w in range(28):
                pw = ps_o.tile([1, TCH], F32, tag="o", name=f"warm{w}")
                nc.tensor.matmul(pw[:], ones5[0:128, 0:1], ones5[:],
                                 start=True, stop=True)

            xT_db = [[const.tile([128, 4, TCH], BF16, name=f"xT{i}_{hf}")
                      for hf in range(2)] for i in range(2)]

            def emit_dma_xt(tcH):
                t0 = tcH * TCH
                for hf in range(2):
                    nc.sync.dma_start(
                        xT_db[tcH % 2][hf][:],
                        x_t[hf * 512:(hf + 1) * 512, t0:t0 + TCH]
                        .rearrange("(s p) t -> p s t", p=128))

            wqk_sb = const.tile([128, 8, 512], BF16)
            nc.sync.dma_start(
                wqk_sb[:, 0:4, :],
                wqk.ap()[0:512, :].rearrange("(s p) m -> p s m", p=128))
            emit_dma_xt(0)
            nc.sync.dma_start(
                wqk_sb[:, 4:8, :],
                wqk.ap()[512:1024, :].rearrange("(s p) m -> p s m", p=128))
            # spread remaining weight loads across idle DGE queues
            cos_sb = const.tile([128, T], BF16)
            nc.scalar.dma_start(cos_sb[:], cos_in[:, :])
            sin_sb = const.tile([128, T], BF16)
            nc.scalar.dma_start(sin_sb[:], sin_in[:, :])
            bqk_sb = const.tile([128, 4], F32)
            nc.scalar.dma_start(bqk_sb[:], bqk[:, :])
            wv_sb = const.tile([128, 8, 256], BF16)
            nc.gpsimd.dma_start(wv_sb[:],
                                wv.ap().rearrange("(s p) m -> p s m", p=128))
            bvr_sb = const.tile([1, 256], BF16)
            nc.gpsimd.dma_start(bvr_sb[:], bvr[:, :])
            tri_sb = const.tile([128, 128], BF16)
            nc.gpsimd.dma_start(tri_sb[:], tri_in[:, :])
            wp_sb = const.tile([128, 2, 1024], BF16)
            nc.gpsimd.dma_start(wp_sb[:],
                                wp.ap().rearrange("(s p) m -> p s m", p=128))

            # ---- persistent activations ----
            qT_sb = const.tile([64, HPC, T], BF16, name="qT")
            kT_sb = const.tile([64, HPC, T], BF16, name="kT")
            v_sb = const.tile([128, NSLAB, HPC, 65], BF16, name="v")
            nc.vector.memset(v_sb[:, :, :, 64], 1.0)  # ones col, stays forever
            # p=0 tiles are parity-double-buffered so chunk tcH+1's S/exp
            # (p=0) can run while chunk tcH's PV still reads its p=0 tiles
            expS0 = [[const.tile([128, 2, TCH], BF16, name=f"eS0{a}_{j}")
                      for j in range(NSLAB)] for a in range(2)]
            expS1 = [const.tile([128, 2, TCH], BF16, name=f"eS1_{j}")
                     for j in range(NSLAB)]

            def expS(tcH, p):
                return expS0[tcH % 2] if p == 0 else expS1

            def emit_qkv(tcH):
                xTh = xT_db[tcH % 2]
                tcols = slice(tcH * TCH, (tcH + 1) * TCH)
                for m in range(HPC):
                    pqk = ps_a.tile([128, TCH], F32, tag="a")
                    for s in range(8):
                        nc.tensor.matmul(pqk[:],
                                         wqk_sb[:, s, m * 128:(m + 1) * 128],
                                         xTh[s // 4][:, s % 4, :],
                                         start=(s == 0), stop=(s == 7))
                    tQ = work.tile([128, TCH], BF16, tag="tQ")
                    nc.vector.scalar_tensor_tensor(
                        tQ[:], pqk[:], bqk_sb[:, m:m + 1], ones5[:],
                        op0=mybir.AluOpType.add, op1=mybir.AluOpType.mult)
                    tA = work.tile([128, TCH], BF16, tag="tA")
                    tB = work.tile([128, TCH], BF16, tag="tB")
                    nc.gpsimd.tensor_mul(tA[:], tQ[:], cos_sb[:, tcols])
                    for (a0, b0) in ((0, 32), (32, 0), (64, 96), (96, 64)):
                        nc.vector.tensor_mul(tB[a0:a0 + 32, :], tQ[b0:b0 + 32, :],
                                             sin_sb[b0:b0 + 32, tcols])
                    nc.vector.tensor_add(qT_sb[:, m, tcols],
                                         tA[0:64, :], tB[0:64, :])
                    nc.vector.tensor_add(kT_sb[:, m, tcols],
                                         tA[64:128, :], tB[64:128, :])
                for ts in range(4):
                    pv = ps_a.tile([128, 256], F32, tag="a")
                    for s in range(8):
                        nc.tensor.matmul(pv[:],
                                         xTh[s // 4][:, s % 4,
                                                     ts * 128:(ts + 1) * 128],
                                         wv_sb[:, s, :], start=(s == 0),
                                         stop=False)
                    nc.tensor.matmul(pv[:], ones_h[0:1, :], bvr_sb[:],
                                     start=False, stop=True,
                                     skip_group_check=True)
                    sl = tcH * 4 + ts
                    nc.scalar.activation(
                        v_sb[:, sl, :, 0:64],
                        pv[:].rearrange("p (h e) -> p h e", e=64), AF.Copy)

            def emit_s_exp(tcH, p):
                nslabs = 4 * tcH + 4
                for j in range(nslabs):
                    rr = j - 4 * tcH
                    r = max(rr, 0) * 128
                    qs = slice(tcH * TCH + r, (tcH + 1) * TCH)
                    psS = ps_s.tile([128, 2, TCH], F32, tag="S")
                    for hh in range(2):
                        h = 2 * p + hh
                        nc.tensor.matmul(psS[:, hh, r:TCH],
                                         kT_sb[:, h, j * 128:(j + 1) * 128],
                                         qT_sb[:, h, qs],
                                         start=True, stop=(rr < 0))
                        if rr >= 0:
                            nc.tensor.matmul(psS[:, hh, r:r + 128], ident[:],
                                             tri_sb[:], start=False, stop=True,
                                             skip_group_check=True)
                    eS = expS(tcH, p)[j]
                    nc.scalar.activation(eS[:, :, r:TCH], psS[:, :, r:TCH],
                                         AF.Exp, scale=0.125)

            def emit_pv(tcH, yT_ch, p):
                yT_p = yT_ch[p]
                nslabs = 4 * tcH + 4
                for hh in range(2):
                    h = 2 * p + hh
                    po = ps_o.tile([65, TCH], F32, tag="o")
                    for j in range(nslabs):
                        rr = j - 4 * tcH
                        r = max(rr, 0) * 128
                        nc.tensor.matmul(po[:, r:TCH],
                                         v_sb[:, j, h, :],
                                         expS(tcH, p)[j][:, hh, r:TCH],
                                         start=(j == 0),
                                         stop=(j == nslabs - 1))
                    l_r = work.tile([1, TCH], BF16, tag="lr")
                    with nc.allow_low_precision(reason="bf16 1/l"):
                        nc.vector.reciprocal(l_r[:], po[64:65, :])
                    l_b = work.tile([64, TCH], BF16, tag="lb")
                    nc.gpsimd.partition_broadcast(l_b[:], l_r[0:1, :])
                    nc.vector.tensor_mul(yT_p[64 * hh:64 * hh + 64, :],
                                         po[0:64, :], l_b[:])

            def emit_proj(tcH, yT_ch, order=(0, 1), pool=None):
                tcols = slice(tcH * TCH, (tcH + 1) * TCH)
                for ct in range(8):
                    pp = (pool or ps_o).tile([128, TCH], F32,
                                             tag="S" if pool else "o")
                    for i, s in enumerate(order):
                        nc.tensor.matmul(pp[:],
                                         wp_sb[:, s, ct * 128:(ct + 1) * 128],
                                         yT_ch[s][:, :], start=(i == 0),
                                         stop=(i == 1))
                    o_st = ost.tile([128, TCH], BF16, tag="ost")
                    nc.vector.tensor_copy(o_st[:], pp[:])
                    nc.sync.dma_start(out_t[ct * 128:(ct + 1) * 128, tcols],
                                      o_st[:])

            # ---- software-pipelined emission ----
            emit_qkv(0)

            pending_proj = None   # (tcH, yT_ch) deferred into next iteration
            for tcH in range(NCH):
                if tcH + 1 < NCH:
                    emit_dma_xt(tcH + 1)
                if tcH == 0:
                    emit_s_exp(0, 0)
                emit_s_exp(tcH, 1)
                if pending_proj is not None:
                    emit_proj(*pending_proj)
                if tcH + 1 < NCH:
                    emit_qkv(tcH + 1)
                    emit_s_exp(tcH + 1, 0)
                yT_ch = [ytp.tile([128, TCH], BF16, tag=f"yT{s}",
                                  name=f"yT{s}") for s in range(2)]
                emit_pv(tcH, yT_ch, 0)
                emit_pv(tcH, yT_ch, 1)
                pending_proj = (tcH, yT_ch)
            emit_proj(*pending_proj, pool=ps_s)

    nc.compile()
    return nc


def _rope_tables():
    pos = np.arange(T, dtype=np.float32)[:, None]                  # [T, 1]
    i = np.arange(1, HS // 2 + 1, dtype=np.float32)[None]          # [1, 32]
    theta = 1.0 / 10000.0 ** (2.0 * (i - 1.0) / HS)
    ang = pos * theta                                              # [T, 32]
    cos, sin = np.cos(ang).T, np.sin(ang).T                        # [32, T]
    cos_rep = np.tile(cos, (4, 1)).astype(np.float32)              # [128, T]
    sin_sgn = np.concatenate([sin, -sin, sin, -sin], 0).astype(np.float32)
    return cos_rep, sin_sgn


def kernel(x, W_qkv, b_qkv, W_proj, b_proj):
    global last_results
    import ml_dtypes
    from concourse.bass_utils import run_bass_kernel_spmd

    BF = ml_dtypes.bfloat16
    F8 = ml_dtypes.float8_e4m3

    def fold(a):
        # [C, N] -> [128, 4, 2, N]: dim (s, j, k) covers c = 256 s + 128 j + k
        n = a.shape[1]
        return np.ascontiguousarray(
            a.reshape(4, 2, 128, n).transpose(2, 0, 1, 3))

    if "nc" not in _cache:
        _cache["nc"] = _build()
    nc = _cache["nc"]

    x = np.asarray(x, np.float32)
    W_qkv = np.asarray(W_qkv, np.float32)
    b_qkv = np.asarray(b_qkv, np.float32)
    W_proj = np.asarray(W_proj, np.float32)
    b_proj = np.asarray(b_proj, np.float32)

    cos_rep, sin_sgn = _rope_tables()
    tri = np.tril(np.full((128, 128), -1e9, dtype=np.float32), -1)

    in_maps = []
    for core in range(NCORES):
        b, g = core // 4, core % 4
        heads = [4 * g + j for j in range(HPC)]
        qk_cols, bqk_cols = [], []
        for h in heads:
            Wq = W_qkv[:, h * 3 * HS:h * 3 * HS + HS]
            Wk = W_qkv[:, h * 3 * HS + HS:h * 3 * HS + 2 * HS]
            bq = b_qkv[h * 3 * HS:h * 3 * HS + HS]
            bk = b_qkv[h * 3 * HS + HS:h * 3 * HS + 2 * HS]
            qk_cols.append(np.concatenate(
                [Wq[:, 0::2], Wq[:, 1::2], Wk[:, 0::2], Wk[:, 1::2]], axis=1))
            bqk_cols.append(np.concatenate(
                [bq[0::2], bq[1::2], bk[0::2], bk[1::2]]))
        wqk_core = np.concatenate(qk_cols, axis=1)                 # [C, 512]
        bqk_core = np.stack(bqk_cols, axis=1)                      # [128, 4]
        wv_core = np.concatenate(
            [W_qkv[:, h * 3 * HS + 2 * HS:h * 3 * HS + 3 * HS] for h in heads],
            axis=1)                                                # [C, 256]
        bv_core = np.concatenate(
            [b_qkv[h * 3 * HS + 2 * HS:h * 3 * HS + 3 * HS] for h in heads])

        in_maps.append({
            "x_t": np.ascontiguousarray(x[b].T).astype(BF),
            "wqk": np.ascontiguousarray(wqk_core).astype(BF),
            "bqk": np.ascontiguousarray(bqk_core),
            "wv": np.ascontiguousarray(wv_core).astype(BF),
            "bvr": np.ascontiguousarray(bv_core[None, :]).astype(BF),
            "wp": np.ascontiguousarray(W_proj[g * 256:(g + 1) * 256, :]).astype(BF),
            "cos_in": cos_rep.astype(BF),
            "sin_in": sin_sgn.astype(BF),
            "tri_in": tri.astype(BF),
        })

    res = run_bass_kernel_spmd(nc, in_maps, core_ids=list(range(NCORES)))
    last_results = res

    out = np.zeros((B, T, C), dtype=np.float32)
    for core in range(NCORES):
        b = core // 4
        out[b] += res.results[core]["out_t"].astype(np.float32).T
    out += b_proj[None, None, :]
    return out
